# revision 26
# baseline (speedup 1.0000x reference)
"""Two-layer GCN (message passing) on 8 Trainium2 NeuronCores.

Strategy (1D graph partitioning by destination node):
  - Nodes are grouped into 128-node tiles; tiles are dealt across the 8 cores
    (balanced by incident-edge count).  Each core owns P tiles ("positions").
  - Per layer:  h = f @ W  (dense, on owned tiles)  ->  g = dinv * h
    -> AllGather g across cores (replicated node-feature table in HBM)
    -> per owned tile: gather source rows of g (dma_gather), build binary
       one-hot scatter matrices S (DVE iota==col), accumulate Sᵀ·M on the
       TensorEngine into PSUM, then out = relu(dinv * agg + b).
  - GCN normalization norm=dinv[row]*dinv[col] is factored into per-node
    pre/post scales (dinv>0 always, self-loops), so S stays binary.
  - dma_gather indices are int16, so the gathered table is split into two
    halves (<32768 rows each); each tile's edge list is split by source half
    on the host.
"""

import os
import sys

sys.path.insert(0, "/opt/trn_rl_repo")

import numpy as np

import concourse.bass as bass  # noqa: E402
import concourse.bacc as bacc  # noqa: E402
import concourse.mybir as mybir  # noqa: E402
from concourse import tile  # noqa: E402
from concourse.bass_utils import run_bass_kernel_spmd  # noqa: E402

NCORES = 8
D = 128

# gather-table dtype: "bf16" halves gather/allgather bytes (dense matmuls and
# all accumulation stay fp32); "f32" is the exact path.
GATHER_DT = os.environ.get("GCN_GATHER_DT", "bf16")

# Filled by kernel() on each run (ns, from the NTFF profile when tracing).
LAST_EXEC_NS = None
LAST_RESULTS = None


def _np_dt(dtg):
    if dtg == "bf16":
        import ml_dtypes

        return ml_dtypes.bfloat16
    return np.float32


def _mybir_dt(dtg):
    return mybir.dt.bfloat16 if dtg == "bf16" else mybir.dt.float32


def _plan(row, col, n_nodes):
    """Host-side graph preprocessing (index work only + degree normalization).

    ``row``/``col`` must NOT include the self-loops the GCN adds — those are
    handled on-device by an identity-matmul block reading the local g tile
    (contiguous DMA, no gather descriptors).  ``deg``/``dinv`` DO account for
    the self-loop (+1).

    Returns a dict with the compile-time plan (identical across cores) and the
    per-core index/feature-layout arrays.
    """
    P8 = NCORES
    NT = -(-n_nodes // 128)
    NTp = -(-NT // P8) * P8
    NODES_PAD = NTp * 128
    P = NTp // P8  # positions (tiles) per core
    NPT = P * 128  # nodes per core
    HALF = NODES_PAD // 2

    deg = (np.bincount(col, minlength=n_nodes) + 1).astype(np.float64)
    dinv = (1.0 / np.sqrt(deg)).astype(np.float32)

    # deal tiles to cores, largest-first so per-position counts match across cores
    tile_cnt = np.bincount(col >> 7, minlength=NTp)
    order = np.argsort(-tile_cnt, kind="stable")
    tile_core = np.empty(NTp, np.int64)
    rr = np.arange(NTp)
    tile_core[order] = rr % P8

    # A/B half of an edge depends only on the SOURCE's core (cores 0-3 -> A),
    # so per-tile A/B incoming counts are known before positions are chosen.
    et = col >> 7
    src_is_A = tile_core[row >> 7] < (P8 // 2)
    cntA_t = np.bincount(et[src_is_A], minlength=NTp)
    cntB_t = np.bincount(et[~src_is_A], minlength=NTp)

    # choose tile->position within each core to minimize
    # sum_tau ceil(max_c A/128) + ceil(max_c B/128)   (gather padding)
    byA = np.zeros((P8, P), np.int64)
    byB = np.zeros((P8, P), np.int64)
    tid = np.zeros((P8, P), np.int64)
    for c in range(P8):
        tiles_c = np.where(tile_core == c)[0]
        o = np.argsort(-(cntA_t[tiles_c] + cntB_t[tiles_c]), kind="stable")
        tiles_c = tiles_c[o]
        tid[c] = tiles_c
        byA[c] = cntA_t[tiles_c]
        byB[c] = cntB_t[tiles_c]

    def blocks(x):
        return -(-x // 128)

    def total_cost(A, B):
        return int(blocks(A.max(axis=0)).sum() + blocks(B.max(axis=0)).sum())

    rngh = np.random.default_rng(0)
    cur = total_cost(byA, byB)
    n_iter = 60000
    cs = rngh.integers(0, P8, n_iter)
    t1s = rngh.integers(0, P, n_iter)
    t2s = rngh.integers(0, P, n_iter)
    KAm = blocks(byA.max(axis=0))
    KBm = blocks(byB.max(axis=0))
    for it in range(n_iter):
        c, u, w = int(cs[it]), int(t1s[it]), int(t2s[it])
        if u == w:
            continue
        for arr in (byA, byB):
            arr[c, u], arr[c, w] = arr[c, w], arr[c, u]
        tid[c, u], tid[c, w] = tid[c, w], tid[c, u]
        nKAu, nKAw = blocks(byA[:, u].max()), blocks(byA[:, w].max())
        nKBu, nKBw = blocks(byB[:, u].max()), blocks(byB[:, w].max())
        delta = (nKAu + nKAw + nKBu + nKBw) - (
            KAm[u] + KAm[w] + KBm[u] + KBm[w]
        )
        if delta <= 0:
            KAm[u], KAm[w], KBm[u], KBm[w] = nKAu, nKAw, nKBu, nKBw
        else:  # revert
            for arr in (byA, byB):
                arr[c, u], arr[c, w] = arr[c, w], arr[c, u]
            tid[c, u], tid[c, w] = tid[c, w], tid[c, u]

    tile_pos = np.empty(NTp, np.int64)
    for c in range(P8):
        tile_pos[tid[c]] = np.arange(P)

    # node -> permuted id in the AllGather layout [core0 tiles..., core1 tiles...]
    v = np.arange(NODES_PAD)
    tv = v >> 7
    pid = tile_core[tv] * NPT + tile_pos[tv] * 128 + (v & 127)

    # per-edge attributes
    ec = tile_core[et]
    ep = tile_pos[et]
    ecl = (col & 127).astype(np.float32)  # local dst within tile
    esrc = pid[row]
    ehalf = (esrc >= HALF).astype(np.int64)
    eidx = (esrc - ehalf * HALF).astype(np.int16)

    key = (ec * P + ep) * 2 + ehalf
    sidx = np.argsort(key, kind="stable")
    counts = np.bincount(key, minlength=P8 * P * 2).reshape(P8, P, 2)
    starts = np.concatenate([[0], np.cumsum(counts.reshape(-1))])

    K_A = np.maximum(1, -(-counts[:, :, 0].max(axis=0) // 128)).astype(np.int64)
    K_B = np.maximum(1, -(-counts[:, :, 1].max(axis=0) // 128)).astype(np.int64)
    capA = K_A * 128
    capB = K_B * 128
    baseA = np.concatenate([[0], np.cumsum(capA)])
    baseB = np.concatenate([[0], np.cumsum(capB)])
    sumA = int(baseA[-1])
    sumB = int(baseB[-1])
    TOTB = int((K_A + K_B).sum())

    # map (core, pos) -> global tile id
    tiles_cp = np.empty((P8, P), np.int64)
    tiles_cp[tile_core, tile_pos] = np.arange(NTp)

    cores = []
    for c in range(P8):
        idxA = np.zeros(sumA, np.int16)  # pad -> row 0 (valid, masked by S)
        colA = np.full(sumA, -1.0, np.float32)  # pad -> -1 (no one-hot match)
        idxB = np.zeros(sumB, np.int16)
        colB = np.full(sumB, -1.0, np.float32)
        for p_ in range(P):
            g = (c * P + p_) * 2
            s, e = starts[g], starts[g + 1]
            ids = sidx[s:e]
            cnt = e - s
            assert cnt <= capA[p_]
            idxA[baseA[p_] : baseA[p_] + cnt] = eidx[ids]
            colA[baseA[p_] : baseA[p_] + cnt] = ecl[ids]
            s, e = starts[g + 1], starts[g + 2]
            ids = sidx[s:e]
            cnt = e - s
            assert cnt <= capB[p_]
            idxB[baseB[p_] : baseB[p_] + cnt] = eidx[ids]
            colB[baseB[p_] : baseB[p_] + cnt] = ecl[ids]

        # colv: per position, A blocks then B blocks; [128 slots, TOTB blocks]
        pieces = []
        for p_ in range(P):
            pieces.append(colA[baseA[p_] : baseA[p_ + 1]])
            pieces.append(colB[baseB[p_] : baseB[p_ + 1]])
        colv = np.concatenate(pieces).reshape(TOTB, 128).T.copy()

        # dma_gather index layout: idx i -> (partition i%16, column i//16),
        # replicated across the 8 groups of 16 partitions
        def wrap(a):
            w = a.reshape(-1, 16).T  # [16, n/16]
            return np.tile(w, (8, 1)).copy()

        nodes_c = (tiles_cp[c][:, None] * 128 + np.arange(128)[None, :]).reshape(-1)
        cores.append(
            dict(
                idxA=wrap(idxA),
                idxB=wrap(idxB),
                colv=colv,
                nodes=nodes_c,
            )
        )

    return dict(
        NT=NTp,
        P=P,
        NPT=NPT,
        NODES_PAD=NODES_PAD,
        HALF=HALF,
        K_A=K_A,
        K_B=K_B,
        TOTB=TOTB,
        sumA=sumA,
        sumB=sumB,
        dinv=dinv,
        cores=cores,
    )


def _make_groups(K_A, K_B, budget):
    """Greedy grouping of consecutive positions for batched gathers."""
    P = len(K_A)
    groups = []
    t0 = 0
    acc = 0
    for t in range(P):
        kt = K_A[t] + K_B[t]
        if acc > 0 and acc + kt > budget:
            groups.append((t0, t))
            t0 = t
            acc = 0
        acc += kt
    groups.append((t0, P))
    return groups


def _build(plan, dtg):
    """Build + compile the SPMD Bass kernel for the given plan."""
    P = plan["P"]
    NPT = plan["NPT"]
    NODES_PAD = plan["NODES_PAD"]
    HALF = plan["HALF"]
    K_A = plan["K_A"]
    K_B = plan["K_B"]
    TOTB = plan["TOTB"]
    sumA = plan["sumA"]
    sumB = plan["sumB"]
    CA = sumA // 16
    CB = sumB // 16
    DT = _mybir_dt(dtg)
    F32 = mybir.dt.float32
    budget = 96 if dtg == "bf16" else 48
    groups = _make_groups(K_A, K_B, budget)
    baseKA = np.concatenate([[0], np.cumsum(K_A)])
    baseKB = np.concatenate([[0], np.cumsum(K_B)])
    baseKT = np.concatenate([[0], np.cumsum(K_A + K_B)])

    nc = bacc.Bacc("TRN2", target_bir_lowering=False, debug=False, num_devices=NCORES)

    xT = nc.dram_tensor("xT", [NPT, D], F32, kind="ExternalInput")
    W1 = nc.dram_tensor("W1", [D, D], F32, kind="ExternalInput")
    W2 = nc.dram_tensor("W2", [D, D], F32, kind="ExternalInput")
    bias = nc.dram_tensor("bias", [2, D], F32, kind="ExternalInput")
    dinv_c = nc.dram_tensor("dinv_c", [P, 128], F32, kind="ExternalInput")
    iota_in = nc.dram_tensor("iota_in", [128, 128], DT, kind="ExternalInput")
    id_in = nc.dram_tensor("id_in", [128, 128], DT, kind="ExternalInput")
    idxA_in = nc.dram_tensor("idxA", [128, CA], mybir.dt.int16, kind="ExternalInput")
    idxB_in = nc.dram_tensor("idxB", [128, CB], mybir.dt.int16, kind="ExternalInput")
    colv_in = nc.dram_tensor("colv", [128, TOTB], DT, kind="ExternalInput")
    outT = nc.dram_tensor("outT", [NPT, D], F32, kind="ExternalOutput")

    with tile.TileContext(nc) as tc:
        with (
            tc.tile_pool(name="const", bufs=1) as constp,
            tc.tile_pool(name="lhs", bufs=3) as lhsp,
            tc.tile_pool(name="gtile", bufs=3) as gp,
            tc.tile_pool(name="ma", bufs=2) as map_,
            tc.tile_pool(name="mb", bufs=2) as mbp,
            tc.tile_pool(name="s", bufs=2) as sp,
            tc.tile_pool(name="post", bufs=3) as postp,
            tc.tile_pool(name="psh", bufs=2, space="PSUM") as pshp,
            tc.tile_pool(name="pso", bufs=4, space="PSUM") as psop,
            tc.tile_pool(name="dram", bufs=1, space="DRAM") as dram,
        ):
            g_loc = [dram.tile([NPT, D], DT, name=f"g_loc{i}") for i in range(2)]
            g_full = [
                dram.tile([NODES_PAD, D], DT, addr_space="Shared", name=f"g_full{i}")
                for i in range(2)
            ]
            # layer-1 outputs stay in SBUF, one tile per position (per-tile deps)
            f2_tiles = [
                constp.tile([128, 128], F32, name=f"f2_{t}") for t in range(P)
            ]

            # --- constants / setup ---
            w1_sb = constp.tile([128, 128], F32)
            nc.sync.dma_start(w1_sb[:], W1[:])
            w2_sb = constp.tile([128, 128], F32)
            nc.sync.dma_start(w2_sb[:], W2[:])
            bias_sb = constp.tile([128, 2], F32)
            nc.sync.dma_start(bias_sb[:], bias.ap().rearrange("l f -> f l"))
            dinvp_sb = constp.tile([128, P], F32)
            nc.sync.dma_start(dinvp_sb[:], dinv_c.ap().rearrange("p v -> v p"))

            iota_sb = constp.tile([128, 128], DT)
            nc.sync.dma_start(iota_sb[:], iota_in[:])
            id_sb = constp.tile([128, 128], DT)
            nc.sync.dma_start(id_sb[:], id_in[:])
            idxA_sb = constp.tile([128, CA], mybir.dt.int16)
            nc.sync.dma_start(idxA_sb[:], idxA_in[:])
            idxB_sb = constp.tile([128, CB], mybir.dt.int16)
            nc.sync.dma_start(idxB_sb[:], idxB_in[:])
            colv_sb = constp.tile([128, TOTB], DT)
            nc.sync.dma_start(colv_sb[:], colv_in[:])
            # broadcast dinv along partitions: dinvb[p, t*128+v] = dinv[t, v]
            # (partition-step-0 DMA read replicates the row into all partitions)
            dinvb_sb = constp.tile([128, NPT], F32)
            dinv_flat = dinv_c.ap().rearrange("p v -> (p v)")
            nc.sync.dma_start(
                dinvb_sb[:], dinv_flat[None, :].broadcast_to([128, NPT])
            )

            # whole xT in one strided DMA: xT_sb[f, t, v] = xT[t*128+f, v]
            xT_sb = constp.tile([128, P, 128], F32)
            nc.sync.dma_start(xT_sb[:], xT.ap().rearrange("(t f) v -> f t v", f=128))

            for l in range(2):
                w_sb = w1_sb if l == 0 else w2_sb

                # --- dense: g = dinv * (f @ W) ---
                for t in range(P):
                    if l == 0:
                        lhsT_ap = xT_sb[:, t, :]
                    else:
                        lhsT_ap = f2_tiles[t][:]
                    ph = pshp.tile([128, 128], F32)
                    nc.tensor.matmul(ph[:], lhsT=lhsT_ap, rhs=w_sb[:], start=True, stop=True)
                    gt = gp.tile([128, 128], DT)
                    nc.vector.tensor_scalar_mul(gt[:], ph[:], dinvp_sb[:, t : t + 1])
                    nc.sync.dma_start(g_loc[l][t * 128 : (t + 1) * 128, :], gt[:])

                nc.gpsimd.collective_compute(
                    "AllGather",
                    mybir.AluOpType.bypass,
                    replica_groups=[list(range(NCORES))],
                    ins=[g_loc[l].opt()],
                    outs=[g_full[l].opt()],
                )
                gA = g_full[l][0:HALF, :]
                gB = g_full[l][HALF:NODES_PAD, :]

                # --- gather + scatter-matmul + post (big groups first so the
                # drain after the last gather is short) ---
                ordered = sorted(
                    groups, key=lambda g: -int(baseKT[g[1]] - baseKT[g[0]])
                )
                for (t0, t1) in ordered:
                    nA = int(baseKA[t1] - baseKA[t0])
                    nB = int(baseKB[t1] - baseKB[t0])
                    MA = map_.tile([128, nA, 128], DT, tag="ma")
                    MB = mbp.tile([128, nB, 128], DT, tag="mb")
                    if os.environ.get("GCN_NO_GATHER", "0") == "1":
                        nc.vector.memset(MA[:], 0.125)
                        nc.vector.memset(MB[:], 0.125)
                    else:
                        nc.gpsimd.dma_gather(
                            MA[:], gA,
                            idxA_sb[:, int(baseKA[t0]) * 8 : int(baseKA[t1]) * 8],
                            nA * 128, nA * 128, 128, elem_step=128,
                            single_packet=False,
                        )
                        nc.gpsimd.dma_gather(
                            MB[:], gB,
                            idxB_sb[:, int(baseKB[t0]) * 8 : int(baseKB[t1]) * 8],
                            nB * 128, nB * 128, 128, elem_step=128,
                            single_packet=False,
                        )
                    # one wide one-hot build for the whole group:
                    # S[p, k, j] = (iota[p, j] == colv[p, b0+k])
                    nT = int(baseKT[t1] - baseKT[t0])
                    Sg = sp.tile([128, nT, 128], DT, tag="s")
                    iota_b = iota_sb[:, :].unsqueeze(1).broadcast_to([128, nT, 128])
                    colv_b = (
                        colv_sb[:, int(baseKT[t0]) : int(baseKT[t1])]
                        .unsqueeze(2)
                        .broadcast_to([128, nT, 128])
                    )
                    nc.vector.tensor_tensor(
                        Sg[:], iota_b, colv_b, op=mybir.AluOpType.is_equal
                    )
                    for t in range(t0, t1):
                        po = psop.tile([128, 128], F32)
                        nblk = int(K_A[t] + K_B[t]) + 1
                        # self-loop term: psum += g_localᵀ (contiguous rows, no gather)
                        ms = lhsp.tile([128, 128], DT, tag="mself")
                        nc.sync.dma_start(
                            ms[:], g_loc[l][t * 128 : (t + 1) * 128, :]
                        )
                        nc.tensor.matmul(
                            po[:], lhsT=ms[:], rhs=id_sb[:],
                            start=True, stop=(nblk == 1),
                        )
                        i = 1
                        sb_a = int(baseKT[t] - baseKT[t0])
                        for k in range(int(K_A[t])):
                            ja = int(baseKA[t] - baseKA[t0]) + k
                            nc.tensor.matmul(
                                po[:], lhsT=MA[:, ja, :], rhs=Sg[:, sb_a + k, :],
                                start=(i == 0), stop=(i == nblk - 1),
                            )
                            i += 1
                        sb_b = sb_a + int(K_A[t])
                        for k in range(int(K_B[t])):
                            jb = int(baseKB[t] - baseKB[t0]) + k
                            nc.tensor.matmul(
                                po[:], lhsT=MB[:, jb, :], rhs=Sg[:, sb_b + k, :],
                                start=(i == 0), stop=(i == nblk - 1),
                            )
                            i += 1
                        tmp = postp.tile([128, 128], F32, tag="tmp")
                        nc.vector.tensor_mul(
                            tmp[:], po[:], dinvb_sb[:, t * 128 : (t + 1) * 128]
                        )
                        if l == 0:
                            nc.scalar.activation(
                                f2_tiles[t][:], tmp[:],
                                mybir.ActivationFunctionType.Relu,
                                bias=bias_sb[:, l : l + 1],
                            )
                        else:
                            ot = postp.tile([128, 128], F32, tag="ot")
                            nc.scalar.activation(
                                ot[:], tmp[:], mybir.ActivationFunctionType.Relu,
                                bias=bias_sb[:, l : l + 1],
                            )
                            nc.sync.dma_start(
                                outT[t * 128 : (t + 1) * 128, :], ot[:]
                            )

    nc.compile()
    return nc


_BUILD_CACHE = {}


def _get_kernel(plan, dtg):
    key = (plan["P"], plan["NODES_PAD"], tuple(plan["K_A"]), tuple(plan["K_B"]), dtg)
    if key not in _BUILD_CACHE:
        _BUILD_CACHE[key] = _build(plan, dtg)
    return _BUILD_CACHE[key]


def kernel(x, edge_index, W1, b1, W2, b2):
    global LAST_EXEC_NS, LAST_RESULTS
    x = np.asarray(x, dtype=np.float32)
    edge_index = np.asarray(edge_index)
    W1 = np.asarray(W1, dtype=np.float32)
    W2 = np.asarray(W2, dtype=np.float32)
    b1 = np.asarray(b1, dtype=np.float32)
    b2 = np.asarray(b2, dtype=np.float32)
    n = x.shape[0]
    dtg = GATHER_DT

    row = edge_index[0].astype(np.int64)
    col = edge_index[1].astype(np.int64)

    plan = _plan(row, col, n)
    nc = _get_kernel(plan, dtg)

    np_dt = _np_dt(dtg)
    P = plan["P"]
    NPT = plan["NPT"]
    NODES_PAD = plan["NODES_PAD"]

    x_pad = np.zeros((NODES_PAD, D), np.float32)
    x_pad[:n] = x
    dinv_pad = np.zeros(NODES_PAD, np.float32)
    dinv_pad[:n] = plan["dinv"]
    iota = np.broadcast_to(np.arange(128, dtype=np.float32), (128, 128)).astype(np_dt)
    ident = np.eye(128, dtype=np.float32).astype(np_dt)
    bias2 = np.stack([b1, b2]).astype(np.float32)

    in_maps = []
    for c in range(NCORES):
        cc = plan["cores"][c]
        nodes = cc["nodes"]
        xT_c = (
            x_pad[nodes].reshape(P, 128, D).transpose(0, 2, 1).reshape(NPT, D).copy()
        )
        in_maps.append(
            {
                "xT": xT_c,
                "W1": W1,
                "W2": W2,
                "bias": bias2,
                "dinv_c": dinv_pad[nodes].reshape(P, 128).copy(),
                "iota_in": np.ascontiguousarray(iota),
                "id_in": ident,
                "idxA": cc["idxA"],
                "idxB": cc["idxB"],
                "colv": cc["colv"].astype(np_dt),
            }
        )

    trace = bool(int(os.environ.get("GCN_TRACE", "0")))
    res = run_bass_kernel_spmd(
        nc, in_maps, list(range(NCORES)), trace=trace
    )
    LAST_EXEC_NS = res.exec_time_ns
    LAST_RESULTS = res

    out = np.zeros((NODES_PAD, D), np.float32)
    for c in range(NCORES):
        o = res.results[c]["outT"]
        o = o.reshape(P, D, 128).transpose(0, 2, 1).reshape(NPT, D)
        out[plan["cores"][c]["nodes"]] = o
    return out[:n]


# revision 32
# speedup vs baseline: 1.0652x; 1.0652x over previous
"""Two-layer GCN (message passing) on 8 Trainium2 NeuronCores.

Strategy (1D graph partitioning by destination node):
  - Nodes are grouped into 128-node tiles; tiles are dealt across the 8 cores
    (balanced by incident-edge count).  Each core owns P tiles ("positions").
  - Per layer:  h = f @ W  (dense, on owned tiles)  ->  g = dinv * h
    -> AllGather g across cores (replicated node-feature table in HBM)
    -> per owned tile: gather source rows of g (dma_gather), build binary
       one-hot scatter matrices S (DVE iota==col), accumulate Sᵀ·M on the
       TensorEngine into PSUM, then out = relu(dinv * agg + b).
  - GCN normalization norm=dinv[row]*dinv[col] is factored into per-node
    pre/post scales (dinv>0 always, self-loops), so S stays binary.
  - dma_gather indices are int16, so the gathered table is split into two
    halves (<32768 rows each); each tile's edge list is split by source half
    on the host.
"""

import os
import sys

sys.path.insert(0, "/opt/trn_rl_repo")

import numpy as np

import concourse.bass as bass  # noqa: E402
import concourse.bacc as bacc  # noqa: E402
import concourse.mybir as mybir  # noqa: E402
from concourse import tile  # noqa: E402
from concourse.bass_utils import run_bass_kernel_spmd  # noqa: E402

NCORES = 8
D = 128

# gather-table dtype: "bf16" halves gather/allgather bytes (dense matmuls and
# all accumulation stay fp32); "f32" is the exact path.
GATHER_DT = os.environ.get("GCN_GATHER_DT", "bf16")

# Filled by kernel() on each run (ns, from the NTFF profile when tracing).
LAST_EXEC_NS = None
LAST_RESULTS = None


def _np_dt(dtg):
    if dtg == "bf16":
        import ml_dtypes

        return ml_dtypes.bfloat16
    return np.float32


def _mybir_dt(dtg):
    return mybir.dt.bfloat16 if dtg == "bf16" else mybir.dt.float32


def _plan(row, col, n_nodes):
    """Host-side graph preprocessing (index work only + degree normalization).

    ``row``/``col`` must NOT include the self-loops the GCN adds — those are
    handled on-device by an identity-matmul block reading the local g tile
    (contiguous DMA, no gather descriptors).  ``deg``/``dinv`` DO account for
    the self-loop (+1).

    Returns a dict with the compile-time plan (identical across cores) and the
    per-core index/feature-layout arrays.
    """
    P8 = NCORES
    NT = -(-n_nodes // 128)
    NTp = -(-NT // P8) * P8
    NODES_PAD = NTp * 128
    P = NTp // P8  # positions (tiles) per core
    NPT = P * 128  # nodes per core
    PA = (P + 1) // 2  # positions in the "A" half (AllGathered first)
    PB = P - PA
    ROWS_A = P8 * PA * 128  # must stay < 32768 for int16 gather indices
    ROWS_B = P8 * PB * 128
    assert ROWS_A < 32768 and ROWS_B < 32768

    deg = (np.bincount(col, minlength=n_nodes) + 1).astype(np.float64)
    dinv = (1.0 / np.sqrt(deg)).astype(np.float32)

    # deal tiles to cores, largest-first so per-position counts match across cores
    tile_cnt = np.bincount(col >> 7, minlength=NTp)
    order = np.argsort(-tile_cnt, kind="stable")
    tile_core = np.empty(NTp, np.int64)
    tile_pos = np.empty(NTp, np.int64)
    rr = np.arange(NTp)
    tile_core[order] = rr % P8
    tile_pos[order] = rr // P8

    # A/B half of an edge = which position-half its SOURCE tile sits in.
    # The initial deal fixes which tiles are A-side; the position hill-climb
    # below only swaps tiles within a side, so halves stay valid.
    et = col >> 7
    src_is_A = tile_pos[row >> 7] < PA
    cntA_t = np.bincount(et[src_is_A], minlength=NTp)
    cntB_t = np.bincount(et[~src_is_A], minlength=NTp)

    # choose tile->position within each (core, side) to minimize
    # sum_tau ceil(max_c A/128) + ceil(max_c B/128)   (gather padding)
    byA = np.zeros((P8, P), np.int64)
    byB = np.zeros((P8, P), np.int64)
    tid = np.zeros((P8, P), np.int64)
    for c in range(P8):
        for lo, hi in ((0, PA), (PA, P)):
            tiles_c = np.where((tile_core == c) & (tile_pos >= lo) & (tile_pos < hi))[0]
            o = np.argsort(-(cntA_t[tiles_c] + cntB_t[tiles_c]), kind="stable")
            tiles_c = tiles_c[o]
            tid[c, lo:hi] = tiles_c
            byA[c, lo:hi] = cntA_t[tiles_c]
            byB[c, lo:hi] = cntB_t[tiles_c]

    def blocks(x):
        return -(-x // 128)

    rngh = np.random.default_rng(0)
    n_iter = 60000
    cs = rngh.integers(0, P8, n_iter)
    sides = rngh.integers(0, 2, n_iter)
    u1 = rngh.integers(0, P, n_iter)
    u2 = rngh.integers(0, P, n_iter)
    KAm = blocks(byA.max(axis=0))
    KBm = blocks(byB.max(axis=0))
    for it in range(n_iter):
        c = int(cs[it])
        if sides[it] == 0:
            u, w = int(u1[it]) % PA, int(u2[it]) % PA
        else:
            u, w = PA + int(u1[it]) % PB, PA + int(u2[it]) % PB
        if u == w:
            continue
        for arr in (byA, byB):
            arr[c, u], arr[c, w] = arr[c, w], arr[c, u]
        tid[c, u], tid[c, w] = tid[c, w], tid[c, u]
        nKAu, nKAw = blocks(byA[:, u].max()), blocks(byA[:, w].max())
        nKBu, nKBw = blocks(byB[:, u].max()), blocks(byB[:, w].max())
        delta = (nKAu + nKAw + nKBu + nKBw) - (
            KAm[u] + KAm[w] + KBm[u] + KBm[w]
        )
        if delta <= 0:
            KAm[u], KAm[w], KBm[u], KBm[w] = nKAu, nKAw, nKBu, nKBw
        else:  # revert
            for arr in (byA, byB):
                arr[c, u], arr[c, w] = arr[c, w], arr[c, u]
            tid[c, u], tid[c, w] = tid[c, w], tid[c, u]

    for c in range(P8):
        tile_pos[tid[c]] = np.arange(P)

    # node -> row in the half-table AllGather layouts:
    #   A: [core0 pos0..PA-1, core1 ..., ...]   B: likewise for pos PA..P-1
    v = np.arange(NODES_PAD)
    tv = v >> 7
    in_A = tile_pos[tv] < PA
    pid = np.where(
        in_A,
        (tile_core[tv] * PA + tile_pos[tv]) * 128 + (v & 127),
        (tile_core[tv] * PB + (tile_pos[tv] - PA)) * 128 + (v & 127),
    )

    # per-edge attributes
    ec = tile_core[et]
    ep = tile_pos[et]
    ecl = (col & 127).astype(np.float32)  # local dst within tile
    ehalf = (~src_is_A).astype(np.int64)
    eidx = pid[row].astype(np.int16)

    key = (ec * P + ep) * 2 + ehalf
    sidx = np.argsort(key, kind="stable")
    counts = np.bincount(key, minlength=P8 * P * 2).reshape(P8, P, 2)
    starts = np.concatenate([[0], np.cumsum(counts.reshape(-1))])

    K_A = np.maximum(1, -(-counts[:, :, 0].max(axis=0) // 128)).astype(np.int64)
    K_B = np.maximum(1, -(-counts[:, :, 1].max(axis=0) // 128)).astype(np.int64)
    capA = K_A * 128
    capB = K_B * 128
    baseA = np.concatenate([[0], np.cumsum(capA)])
    baseB = np.concatenate([[0], np.cumsum(capB)])
    sumA = int(baseA[-1])
    sumB = int(baseB[-1])
    TOTB = int((K_A + K_B).sum())

    # map (core, pos) -> global tile id
    tiles_cp = np.empty((P8, P), np.int64)
    tiles_cp[tile_core, tile_pos] = np.arange(NTp)

    cores = []
    for c in range(P8):
        idxA = np.zeros(sumA, np.int16)  # pad -> row 0 (valid, masked by S)
        colA = np.full(sumA, -1.0, np.float32)  # pad -> -1 (no one-hot match)
        idxB = np.zeros(sumB, np.int16)
        colB = np.full(sumB, -1.0, np.float32)
        for p_ in range(P):
            g = (c * P + p_) * 2
            s, e = starts[g], starts[g + 1]
            ids = sidx[s:e]
            cnt = e - s
            assert cnt <= capA[p_]
            idxA[baseA[p_] : baseA[p_] + cnt] = eidx[ids]
            colA[baseA[p_] : baseA[p_] + cnt] = ecl[ids]
            s, e = starts[g + 1], starts[g + 2]
            ids = sidx[s:e]
            cnt = e - s
            assert cnt <= capB[p_]
            idxB[baseB[p_] : baseB[p_] + cnt] = eidx[ids]
            colB[baseB[p_] : baseB[p_] + cnt] = ecl[ids]

        # colv: per position, A blocks then B blocks; [128 slots, TOTB blocks]
        pieces = []
        for p_ in range(P):
            pieces.append(colA[baseA[p_] : baseA[p_ + 1]])
            pieces.append(colB[baseB[p_] : baseB[p_ + 1]])
        colv = np.concatenate(pieces).reshape(TOTB, 128).T.copy()

        # dma_gather index layout: idx i -> (partition i%16, column i//16),
        # replicated across the 8 groups of 16 partitions
        def wrap(a):
            w = a.reshape(-1, 16).T  # [16, n/16]
            return np.tile(w, (8, 1)).copy()

        nodes_c = (tiles_cp[c][:, None] * 128 + np.arange(128)[None, :]).reshape(-1)
        cores.append(
            dict(
                idxA=wrap(idxA),
                idxB=wrap(idxB),
                colv=colv,
                nodes=nodes_c,
            )
        )

    return dict(
        NT=NTp,
        P=P,
        PA=PA,
        PB=PB,
        ROWS_A=ROWS_A,
        ROWS_B=ROWS_B,
        NPT=NPT,
        NODES_PAD=NODES_PAD,
        K_A=K_A,
        K_B=K_B,
        TOTB=TOTB,
        sumA=sumA,
        sumB=sumB,
        dinv=dinv,
        cores=cores,
    )


def _make_groups(K_A, K_B, budget):
    """Greedy grouping of consecutive positions for batched gathers."""
    P = len(K_A)
    groups = []
    t0 = 0
    acc = 0
    for t in range(P):
        kt = K_A[t] + K_B[t]
        if acc > 0 and acc + kt > budget:
            groups.append((t0, t))
            t0 = t
            acc = 0
        acc += kt
    groups.append((t0, P))
    return groups


def _build(plan, dtg):
    """Build + compile the SPMD Bass kernel for the given plan."""
    P = plan["P"]
    NPT = plan["NPT"]
    NODES_PAD = plan["NODES_PAD"]
    PA, PB = plan["PA"], plan["PB"]
    ROWS_A, ROWS_B = plan["ROWS_A"], plan["ROWS_B"]
    K_A = plan["K_A"]
    K_B = plan["K_B"]
    TOTB = plan["TOTB"]
    sumA = plan["sumA"]
    sumB = plan["sumB"]
    CA = sumA // 16
    CB = sumB // 16
    DT = _mybir_dt(dtg)
    F32 = mybir.dt.float32
    budget = 96 if dtg == "bf16" else 48
    groups = _make_groups(K_A, K_B, budget)
    baseKA = np.concatenate([[0], np.cumsum(K_A)])
    baseKB = np.concatenate([[0], np.cumsum(K_B)])
    baseKT = np.concatenate([[0], np.cumsum(K_A + K_B)])

    nc = bacc.Bacc("TRN2", target_bir_lowering=False, debug=False, num_devices=NCORES)

    xT = nc.dram_tensor("xT", [NPT, D], F32, kind="ExternalInput")
    W1 = nc.dram_tensor("W1", [D, D], F32, kind="ExternalInput")
    W2 = nc.dram_tensor("W2", [D, D], F32, kind="ExternalInput")
    bias = nc.dram_tensor("bias", [2, D], F32, kind="ExternalInput")
    dinv_c = nc.dram_tensor("dinv_c", [P, 128], F32, kind="ExternalInput")
    iota_in = nc.dram_tensor("iota_in", [128, 128], DT, kind="ExternalInput")
    id_in = nc.dram_tensor("id_in", [128, 128], DT, kind="ExternalInput")
    idxA_in = nc.dram_tensor("idxA", [128, CA], mybir.dt.int16, kind="ExternalInput")
    idxB_in = nc.dram_tensor("idxB", [128, CB], mybir.dt.int16, kind="ExternalInput")
    colv_in = nc.dram_tensor("colv", [128, TOTB], DT, kind="ExternalInput")
    outT = nc.dram_tensor("outT", [NPT, D], F32, kind="ExternalOutput")

    with tile.TileContext(nc) as tc:
        with (
            tc.tile_pool(name="const", bufs=1) as constp,
            tc.tile_pool(name="lhs", bufs=3) as lhsp,
            tc.tile_pool(name="gtile", bufs=3) as gp,
            tc.tile_pool(name="ma", bufs=2) as map_,
            tc.tile_pool(name="mb", bufs=2) as mbp,
            tc.tile_pool(name="s", bufs=2) as sp,
            tc.tile_pool(name="post", bufs=3) as postp,
            tc.tile_pool(name="psh", bufs=2, space="PSUM") as pshp,
            tc.tile_pool(name="pso", bufs=4, space="PSUM") as psop,
            tc.tile_pool(name="dram", bufs=1, space="DRAM") as dram,
        ):
            g_locA = [dram.tile([PA * 128, D], DT, name=f"g_locA{i}") for i in range(2)]
            g_locB = [dram.tile([PB * 128, D], DT, name=f"g_locB{i}") for i in range(2)]
            g_fullA = [
                dram.tile([ROWS_A, D], DT, addr_space="Shared", name=f"g_fullA{i}")
                for i in range(2)
            ]
            g_fullB = [
                dram.tile([ROWS_B, D], DT, addr_space="Shared", name=f"g_fullB{i}")
                for i in range(2)
            ]
            # layer-1 outputs stay in SBUF, one tile per position (per-tile deps)
            f2_tiles = [
                constp.tile([128, 128], F32, name=f"f2_{t}") for t in range(P)
            ]

            # --- constants / setup ---
            w1_sb = constp.tile([128, 128], F32)
            nc.sync.dma_start(w1_sb[:], W1[:])
            w2_sb = constp.tile([128, 128], F32)
            nc.sync.dma_start(w2_sb[:], W2[:])
            bias_sb = constp.tile([128, 2], F32)
            nc.sync.dma_start(bias_sb[:], bias.ap().rearrange("l f -> f l"))
            dinvp_sb = constp.tile([128, P], F32)
            nc.sync.dma_start(dinvp_sb[:], dinv_c.ap().rearrange("p v -> v p"))

            iota_sb = constp.tile([128, 128], DT)
            nc.sync.dma_start(iota_sb[:], iota_in[:])
            id_sb = constp.tile([128, 128], DT)
            nc.sync.dma_start(id_sb[:], id_in[:])
            idxA_sb = constp.tile([128, CA], mybir.dt.int16)
            nc.sync.dma_start(idxA_sb[:], idxA_in[:])
            idxB_sb = constp.tile([128, CB], mybir.dt.int16)
            nc.sync.dma_start(idxB_sb[:], idxB_in[:])
            colv_sb = constp.tile([128, TOTB], DT)
            nc.sync.dma_start(colv_sb[:], colv_in[:])
            # broadcast dinv along partitions: dinvb[p, t*128+v] = dinv[t, v]
            # (partition-step-0 DMA read replicates the row into all partitions)
            dinvb_sb = constp.tile([128, NPT], F32)
            dinv_flat = dinv_c.ap().rearrange("p v -> (p v)")
            nc.sync.dma_start(
                dinvb_sb[:], dinv_flat[None, :].broadcast_to([128, NPT])
            )

            # whole xT in one strided DMA: xT_sb[f, t, v] = xT[t*128+f, v]
            xT_sb = constp.tile([128, P, 128], F32)
            nc.sync.dma_start(xT_sb[:], xT.ap().rearrange("(t f) v -> f t v", f=128))

            for l in range(2):
                w_sb = w1_sb if l == 0 else w2_sb

                # --- dense: g = dinv * (f @ W); AllGather the A-half as soon
                # as its positions are done so A-gathers can start early ---
                for t in range(P):
                    if l == 0:
                        lhsT_ap = xT_sb[:, t, :]
                    else:
                        lhsT_ap = f2_tiles[t][:]
                    ph = pshp.tile([128, 128], F32)
                    nc.tensor.matmul(ph[:], lhsT=lhsT_ap, rhs=w_sb[:], start=True, stop=True)
                    gt = gp.tile([128, 128], DT)
                    nc.vector.tensor_scalar_mul(gt[:], ph[:], dinvp_sb[:, t : t + 1])
                    if t < PA:
                        nc.sync.dma_start(
                            g_locA[l][t * 128 : (t + 1) * 128, :], gt[:]
                        )
                    else:
                        nc.sync.dma_start(
                            g_locB[l][(t - PA) * 128 : (t - PA + 1) * 128, :], gt[:]
                        )
                    if t == PA - 1:
                        nc.gpsimd.collective_compute(
                            "AllGather",
                            mybir.AluOpType.bypass,
                            replica_groups=[list(range(NCORES))],
                            ins=[g_locA[l].opt()],
                            outs=[g_fullA[l].opt()],
                        )
                nc.gpsimd.collective_compute(
                    "AllGather",
                    mybir.AluOpType.bypass,
                    replica_groups=[list(range(NCORES))],
                    ins=[g_locB[l].opt()],
                    outs=[g_fullB[l].opt()],
                )
                gA = g_fullA[l][:, :]
                gB = g_fullB[l][:, :]

                # --- gather + scatter-matmul + post (big groups first so the
                # drain after the last gather is short) ---
                ordered = sorted(
                    groups, key=lambda g: -int(baseKT[g[1]] - baseKT[g[0]])
                )
                for (t0, t1) in ordered:
                    nA = int(baseKA[t1] - baseKA[t0])
                    nB = int(baseKB[t1] - baseKB[t0])
                    MA = map_.tile([128, nA, 128], DT, tag="ma")
                    MB = mbp.tile([128, nB, 128], DT, tag="mb")
                    if os.environ.get("GCN_NO_GATHER", "0") == "1":
                        nc.vector.memset(MA[:], 0.125)
                        nc.vector.memset(MB[:], 0.125)
                    else:
                        nc.gpsimd.dma_gather(
                            MA[:], gA,
                            idxA_sb[:, int(baseKA[t0]) * 8 : int(baseKA[t1]) * 8],
                            nA * 128, nA * 128, 128, elem_step=128,
                            single_packet=False,
                        )
                        nc.gpsimd.dma_gather(
                            MB[:], gB,
                            idxB_sb[:, int(baseKB[t0]) * 8 : int(baseKB[t1]) * 8],
                            nB * 128, nB * 128, 128, elem_step=128,
                            single_packet=False,
                        )
                    # one wide one-hot build for the whole group:
                    # S[p, k, j] = (iota[p, j] == colv[p, b0+k])
                    nT = int(baseKT[t1] - baseKT[t0])
                    Sg = sp.tile([128, nT, 128], DT, tag="s")
                    iota_b = iota_sb[:, :].unsqueeze(1).broadcast_to([128, nT, 128])
                    colv_b = (
                        colv_sb[:, int(baseKT[t0]) : int(baseKT[t1])]
                        .unsqueeze(2)
                        .broadcast_to([128, nT, 128])
                    )
                    nc.vector.tensor_tensor(
                        Sg[:], iota_b, colv_b, op=mybir.AluOpType.is_equal
                    )
                    for t in range(t0, t1):
                        po = psop.tile([128, 128], F32)
                        nblk = int(K_A[t] + K_B[t]) + 1
                        # self-loop term: psum += g_localᵀ (contiguous rows, no gather)
                        ms = lhsp.tile([128, 128], DT, tag="mself")
                        if t < PA:
                            ms_src = g_locA[l][t * 128 : (t + 1) * 128, :]
                        else:
                            ms_src = g_locB[l][(t - PA) * 128 : (t - PA + 1) * 128, :]
                        nc.sync.dma_start(ms[:], ms_src)
                        nc.tensor.matmul(
                            po[:], lhsT=ms[:], rhs=id_sb[:],
                            start=True, stop=(nblk == 1),
                        )
                        i = 1
                        sb_a = int(baseKT[t] - baseKT[t0])
                        for k in range(int(K_A[t])):
                            ja = int(baseKA[t] - baseKA[t0]) + k
                            nc.tensor.matmul(
                                po[:], lhsT=MA[:, ja, :], rhs=Sg[:, sb_a + k, :],
                                start=(i == 0), stop=(i == nblk - 1),
                            )
                            i += 1
                        sb_b = sb_a + int(K_A[t])
                        for k in range(int(K_B[t])):
                            jb = int(baseKB[t] - baseKB[t0]) + k
                            nc.tensor.matmul(
                                po[:], lhsT=MB[:, jb, :], rhs=Sg[:, sb_b + k, :],
                                start=(i == 0), stop=(i == nblk - 1),
                            )
                            i += 1
                        tmp = postp.tile([128, 128], F32, tag="tmp")
                        nc.vector.tensor_mul(
                            tmp[:], po[:], dinvb_sb[:, t * 128 : (t + 1) * 128]
                        )
                        if l == 0:
                            nc.scalar.activation(
                                f2_tiles[t][:], tmp[:],
                                mybir.ActivationFunctionType.Relu,
                                bias=bias_sb[:, l : l + 1],
                            )
                        else:
                            ot = postp.tile([128, 128], F32, tag="ot")
                            nc.scalar.activation(
                                ot[:], tmp[:], mybir.ActivationFunctionType.Relu,
                                bias=bias_sb[:, l : l + 1],
                            )
                            nc.sync.dma_start(
                                outT[t * 128 : (t + 1) * 128, :], ot[:]
                            )

    nc.compile()
    return nc


_BUILD_CACHE = {}


def _get_kernel(plan, dtg):
    key = (plan["P"], plan["NODES_PAD"], tuple(plan["K_A"]), tuple(plan["K_B"]), dtg)
    if key not in _BUILD_CACHE:
        _BUILD_CACHE[key] = _build(plan, dtg)
    return _BUILD_CACHE[key]


def kernel(x, edge_index, W1, b1, W2, b2):
    global LAST_EXEC_NS, LAST_RESULTS
    x = np.asarray(x, dtype=np.float32)
    edge_index = np.asarray(edge_index)
    W1 = np.asarray(W1, dtype=np.float32)
    W2 = np.asarray(W2, dtype=np.float32)
    b1 = np.asarray(b1, dtype=np.float32)
    b2 = np.asarray(b2, dtype=np.float32)
    n = x.shape[0]
    dtg = GATHER_DT

    row = edge_index[0].astype(np.int64)
    col = edge_index[1].astype(np.int64)

    plan = _plan(row, col, n)
    nc = _get_kernel(plan, dtg)

    np_dt = _np_dt(dtg)
    P = plan["P"]
    NPT = plan["NPT"]
    NODES_PAD = plan["NODES_PAD"]

    x_pad = np.zeros((NODES_PAD, D), np.float32)
    x_pad[:n] = x
    dinv_pad = np.zeros(NODES_PAD, np.float32)
    dinv_pad[:n] = plan["dinv"]
    iota = np.broadcast_to(np.arange(128, dtype=np.float32), (128, 128)).astype(np_dt)
    ident = np.eye(128, dtype=np.float32).astype(np_dt)
    bias2 = np.stack([b1, b2]).astype(np.float32)

    in_maps = []
    for c in range(NCORES):
        cc = plan["cores"][c]
        nodes = cc["nodes"]
        xT_c = (
            x_pad[nodes].reshape(P, 128, D).transpose(0, 2, 1).reshape(NPT, D).copy()
        )
        in_maps.append(
            {
                "xT": xT_c,
                "W1": W1,
                "W2": W2,
                "bias": bias2,
                "dinv_c": dinv_pad[nodes].reshape(P, 128).copy(),
                "iota_in": np.ascontiguousarray(iota),
                "id_in": ident,
                "idxA": cc["idxA"],
                "idxB": cc["idxB"],
                "colv": cc["colv"].astype(np_dt),
            }
        )

    trace = bool(int(os.environ.get("GCN_TRACE", "0")))
    res = run_bass_kernel_spmd(
        nc, in_maps, list(range(NCORES)), trace=trace
    )
    LAST_EXEC_NS = res.exec_time_ns
    LAST_RESULTS = res

    out = np.zeros((NODES_PAD, D), np.float32)
    for c in range(NCORES):
        o = res.results[c]["outT"]
        o = o.reshape(P, D, 128).transpose(0, 2, 1).reshape(NPT, D)
        out[plan["cores"][c]["nodes"]] = o
    return out[:n]
